# revision 15
# baseline (speedup 1.0000x reference)
"""Single-head attention (no 1/sqrt(d) scaling) for Trainium2, 8 NeuronCores.

Problem: x [8, 2048, 768], W [2304, 768], b [2304]
    qkv = x @ W.T + b ; q,k,v = split(qkv)
    out = softmax(q @ k.T) @ v            -> [8, 2048, 768] fp32

Sharding: data-parallel over batch, one batch element per core. Inputs are
host-transposed (xT [768,2048], wT [768,2304]); the kernel emits out^T
[768, 2048] and the host transposes back during the gather.

Projections and the QK^T logits run in fp32r (full PE rate, ~1.5e-4 rel
rounding — bf16 upstream of the softmax blows up the logit error). The
post-softmax P tiles and v are bf16 (~2e-3 end-to-end rel err, well under
the 2e-2 gate): downstream of exp the quantization no longer amplifies,
and bf16 stationaries get fast weight loads.

Phase A (k/v projection), looped over 512-wide n-slices of x streaming
through double-buffered SBUF slots, W resident:
    kT = (x @ Wk.T + bk).T  transposed layout [h, n]. Resident.
    v  = x @ Wv.T + bv      natural layout, bf16. Resident.
Phase B (attention) per 512-wide n-slice; no max subtraction (|logits| <~60
<< 88 so exp stays in fp32 range; denominators handled unnormalized):
    qT strip = (x @ Wq.T + bq).T  projected on the fly
    S^T[m,n] = k qT       (PSUM, 6 accumulating matmuls per m-chunk)
    P = exp(S^T)          (ACT, rounds to bf16)
    U^T += v_m^T @ P      (6 matmuls, accumulated over 16 m-chunks in 6 banks)
    racc += P             (DVE elementwise accumulate of the 16 P tiles)
    r = ones128 @ racc    (ONE replicated-denominator matmul per slice —
                           replaces the per-m-chunk ones matmul, saving 15
                           512-col PE streams per slice)
    out^T slice = U^T * (1/r)  (approx-reciprocal + DVE scale, DMA to DRAM)
The m-loop is software-pipelined (S/exp for chunk i issued ahead of
racc/U for chunk i-1); S tiles double-buffer through 2 PSUM banks, U 6.

Startup: the PE idles ~10us waiting for the first W/x DMAs while the HAM
clock gate holds it at 1.2 GHz. A dozen dummy matmuls on a memset tile
warm the clock during the DMA window, and the wk/x slice-0 loads are
interleaved so the first real matmul unlocks after ~0.5 MB instead of
~2.7 MB.
"""

import contextlib

import numpy as np

import concourse.bacc as bacc
import concourse.mybir as mybir
import concourse.tile as tile
from concourse.bass_utils import run_bass_kernel_spmd

F32 = mybir.dt.float32
F32R = mybir.dt.float32r
BF16 = mybir.dt.bfloat16
AF = mybir.ActivationFunctionType
ALU = mybir.AluOpType

B, N, H = 8, 2048, 768
H3 = 3 * H
P = 128
ND = H // P      # 6 d-chunks
NM = N // P      # 16 m-chunks
SL = 512         # n-slice width (fp32 moving-operand max / one PSUM bank)
NSL = N // SL    # 4 n-slices


def build_nc(loop_iters=None, split=1, pv_bf16=True, STORE_GP=True, WARM_MMS=16,
             fast_recip=True):
    """Build the attention kernel. loop_iters wraps the whole body in an
    on-device For_i loop (benchmarking only — amortizes dispatch overhead)."""
    nc = bacc.Bacc("TRN2", target_bir_lowering=False, debug=False)

    xT = nc.dram_tensor("xT", [H, N], F32R, kind="ExternalInput")
    wT = nc.dram_tensor("wT", [H, H3], F32R, kind="ExternalInput")
    bcol = nc.dram_tensor("bcol", [P, 2 * ND], F32, kind="ExternalInput")
    bvrow = nc.dram_tensor("bvrow", [1, H], F32, kind="ExternalInput")
    out = nc.dram_tensor("out", [H, N], F32, kind="ExternalOutput")  # transposed; host fixes layout

    def mm_group(psum, lhs_list, rhs_slicer, split=1):
        """Accumulating matmul group into `psum` [P, width]."""
        width = psum.shape[-1]
        hw = width // split
        n = len(lhs_list)
        steps = [(c, h) for c in range(n) for h in range(split)]
        for idx, (c, h) in enumerate(steps):
            lo = h * hw
            nc.tensor.matmul(
                psum[:, lo : lo + hw], lhs_list[c], rhs_slicer(c, lo, hw),
                start=(idx == 0), stop=(idx == len(steps) - 1),
            )

    pdt = BF16 if pv_bf16 else F32R

    with tile.TileContext(nc) as tc:
        with (
            tc.tile_pool(name="dram", bufs=1, space="DRAM") as dram,
            tc.tile_pool(name="const", bufs=1) as const,
            tc.tile_pool(name="keep", bufs=1) as keep,
            tc.For_i(0, loop_iters, 1) if loop_iters else contextlib.nullcontext(),
        ):
            # ---- HAM warmup: dummy matmuls while the first DMAs stream ----
            warm = const.tile([P, SL], F32, name="warm")
            nc.vector.memset(warm[:], 0.0)
            if WARM_MMS:
                warm_ps = tc.alloc_tile_pool(name="warm_ps", bufs=1, space="PSUM")
                wps = warm_ps.tile([P, SL], F32, name="wps")
                for _ in range(WARM_MMS):
                    nc.tensor.matmul(
                        wps[:], warm[:, 0:P].bitcast(F32R), warm[:].bitcast(F32R),
                        start=True, stop=True,
                    )
                warm_ps.release()

            bcol_sb = const.tile([P, 2 * ND], F32)
            ones_f32 = const.tile([P, P], F32, name="ones_f32")
            nc.vector.memset(ones_f32[:], 1.0)
            ones_f32r = ones_f32[:].bitcast(F32R)

            # resident across phases
            ktsb = [keep.tile([P, N], F32R, name=f"kT{c}") for c in range(ND)]
            vsb = [keep.tile([P, H], pdt, name=f"v{ni}") for ni in range(NM)]

            with tc.tile_pool(name="xw_pool", bufs=1) as xw:
                # W resident. q/k sections as [128,128] h-slices so compute
                # unlocks at DMA-stream granularity; v as [128, 768].
                HH = H // 2
                wq = [
                    [xw.tile([P, HH], F32R, name=f"wq{c}_{h}") for h in range(2)]
                    for c in range(ND)
                ]

                def wslice(blks, c, hc):
                    half, col = divmod(hc * P, HH)
                    return blks[c][half][:, col : col + P]
                xwa = tc.alloc_tile_pool(name="xwa_pool", bufs=1)
                wk = [
                    [xwa.tile([P, HH], F32R, name=f"wk{c}_{h}") for h in range(2)]
                    for c in range(ND)
                ]
                wv = [xwa.tile([P, H], F32R, name=f"wv{c}") for c in range(ND)]
                # x is resident for the whole kernel as 2 pairs of
                # [128, 1024] tiles per d-chunk (4KB DMA rows stream the
                # rings at better efficiency than 2KB; phase B reuses the
                # tiles, eliminating the 6MB reload).
                XW = 2 * SL
                xres = [
                    [xw.tile([P, XW], F32R, name=f"x{c}_{pr}") for pr in range(2)]
                    for c in range(ND)
                ]

                def xsl(ns, c, lo, w):
                    return xres[c][ns // 2][:, (ns % 2) * SL + lo : (ns % 2) * SL + lo + w]

                def load_x_pair(pr, interleave_with=None, fine=False):
                    # fine=True loads each tile as two [128, SL] sub-DMAs so
                    # subtile deps unlock the first slice's matmuls per
                    # 256KB instead of per 512KB (startup feed rate).
                    subs = (
                        [(0, SL), (SL, SL)] if fine else [(0, XW)]
                    )
                    for lo, w in subs:
                        for c in range(ND):
                            if interleave_with is not None and lo == 0:
                                interleave_with(c)
                            nc.sync.dma_start(
                                xres[c][pr][:, lo : lo + w],
                                xT.ap()[
                                    c * P : (c + 1) * P,
                                    pr * XW + lo : pr * XW + lo + w,
                                ],
                            )

                def load_w_half(blks, lo, h, c_only=None):
                    for c in range(ND) if c_only is None else [c_only]:
                        nc.sync.dma_start(
                            blks[c][h][:],
                            wT.ap()[c * P : (c + 1) * P, lo + h * HH : lo + (h + 1) * HH],
                        )

                bvb = xwa.tile([P, H], F32, name="bvb")

                # DMA order = compute-unlock order: the first k-projection
                # matmul for chunk c needs wk[c] half0 + x pair-0 chunk c,
                # so those are interleaved pairwise and issued before the
                # small bias loads; wq (phase B only) last.
                load_x_pair(0, interleave_with=lambda c: load_w_half(wk, H, 0, c),
                            fine=True)
                nc.sync.dma_start(bcol_sb[:], bcol.ap())
                nc.sync.dma_start(bvb[:1, :], bvrow.ap())
                nc.gpsimd.partition_broadcast(bvb[:], bvb[:1, :])
                load_w_half(wk, H, 1)
                for c in range(ND):
                    nc.sync.dma_start(
                        wv[c][:], wT.ap()[c * P : (c + 1) * P, 2 * H : 3 * H]
                    )
                load_x_pair(1)
                load_w_half(wq, 0, 0)
                load_w_half(wq, 0, 1)

                with (
                    tc.tile_pool(name="qkps", bufs=3, space="PSUM") as qkps,
                    tc.tile_pool(name="vps", bufs=2, space="PSUM") as vps,
                ):
                    for ns in range(NSL):
                        ssl = slice(ns * SL, (ns + 1) * SL)

                        # --- k projection for this slice (resident) ---
                        for hc in range(ND):
                            ps = qkps.tile([P, SL], F32, name="qkpsum", tag="qk")
                            mm_group(
                                ps, [wslice(wk, c, hc) for c in range(ND)],
                                lambda c, lo, w, _ns=ns: xsl(_ns, c, lo, w),
                                split=split,
                            )
                            nc.scalar.activation(
                                ktsb[hc][:, ssl], ps[:], AF.Identity,
                                bias=bcol_sb[:, ND + hc : ND + hc + 1],
                            )

                        # --- v projection for the 4 n-chunks of this slice ---
                        for ni in range(4 * ns, 4 * ns + 4):
                            lof = (ni % NSL) * P
                            pa = vps.tile([P, SL], F32, name="pa", tag="pa")
                            pb = vps.tile([P, H - SL], F32, name="pb", tag="pb")
                            mm_group(
                                pa, [xsl(ns, c, lof, P) for c in range(ND)],
                                lambda c, lo, w: wv[c][:, lo : lo + w],
                                split=split,
                            )
                            mm_group(
                                pb, [xsl(ns, c, lof, P) for c in range(ND)],
                                lambda c, lo, w: wv[c][:, SL + lo : SL + lo + w],
                            )
                            nc.vector.tensor_tensor(
                                vsb[ni][:, 0:SL], pa[:], bvb[:, 0:SL], op=ALU.add
                            )
                            nc.vector.tensor_tensor(
                                vsb[ni][:, SL:H], pb[:], bvb[:, SL:H], op=ALU.add
                            )

                xwa.release()

                # ---- Phase B: attention (software-pipelined m-loop) ----
                with (
                    tc.tile_pool(name="qsb_pool", bufs=2) as qsb_pool,
                    tc.tile_pool(name="p_pool", bufs=4) as p_pool,
                    tc.tile_pool(name="u_ps", bufs=1, space="PSUM") as u_ps,
                    tc.tile_pool(name="sps", bufs=2, space="PSUM") as sps,
                    tc.tile_pool(name="usb_pool", bufs=1) as usb_pool,
                    tc.tile_pool(name="misc", bufs=1) as misc,
                ):
                    for ns in range(NSL):
                        # project this slice's q strip (transposed layout)
                        qsbuf = []
                        for hc in range(ND):
                            ps = sps.tile([P, SL], F32, name="s_ps", tag="s")
                            mm_group(
                                ps, [wslice(wq, c, hc) for c in range(ND)],
                                lambda c, lo, w, _ns=ns: xsl(_ns, c, lo, w),
                                split=split,
                            )
                            qc = qsb_pool.tile([P, SL], F32R, name=f"qsb{hc}", tag=f"qsb{hc}")
                            nc.scalar.activation(
                                qc[:], ps[:], AF.Identity, bias=bcol_sb[:, hc : hc + 1]
                            )
                            qsbuf.append(qc)
                        us = [
                            u_ps.tile([P, SL], F32, name=f"u{c}", tag=f"u{c}")
                            for c in range(ND)
                        ]
                        racc = misc.tile([P, SL], F32R, name="racc", tag="racc")

                        p_sbs = [None] * NM
                        for mi in range(NM + 1):
                            if mi < NM:
                                msl = slice(mi * P, (mi + 1) * P)
                                s_ps = sps.tile([P, SL], F32, name="s_ps", tag="s")
                                mm_group(
                                    s_ps, [ktsb[c][:, msl] for c in range(ND)],
                                    lambda c, lo, w: qsbuf[c][:, lo : lo + w],
                                    split=split,
                                )
                                p_sb = p_pool.tile([P, SL], pdt, name="p_sb", tag="p")
                                nc.scalar.activation(p_sb[:], s_ps[:], AF.Exp)
                                p_sbs[mi] = p_sb
                            if mi >= 1:
                                j = mi - 1
                                pj = p_sbs[j]
                                if j == 0:
                                    nc.vector.tensor_copy(racc[:], pj[:])
                                else:
                                    nc.vector.tensor_tensor(
                                        racc[:], pj[:], racc[:], op=ALU.add
                                    )
                                for c in range(ND):
                                    hw2 = SL // split
                                    for h in range(split):
                                        lo = h * hw2
                                        nc.tensor.matmul(
                                            us[c][:, lo : lo + hw2],
                                            vsb[j][:, c * P : (c + 1) * P],
                                            pj[:, lo : lo + hw2],
                                            start=(j == 0 and h == 0),
                                            stop=(j == NM - 1 and h == split - 1),
                                        )
                                p_sbs[j] = None

                        # one replicated-denominator matmul for the whole slice
                        r_ps = sps.tile([P, SL], F32, name="r_ps", tag="s")
                        nc.tensor.matmul(
                            r_ps[:], ones_f32r, racc[:], start=True, stop=True
                        )
                        rinv = misc.tile([P, SL], F32, name="rinv", tag="rinv")
                        if fast_recip:
                            nc.vector.reciprocal_approx_fast(rinv[:], r_ps[:])
                        else:
                            nc.vector.reciprocal(rinv[:], r_ps[:])

                        for c in range(ND):
                            u_sb = usb_pool.tile([P, SL], F32, name=f"usb{c}", tag=f"usb{c}")
                            nc.vector.tensor_tensor(u_sb[:], us[c][:], rinv[:], op=ALU.mult)
                            store_eng = nc.gpsimd if STORE_GP else nc.sync
                            store_eng.dma_start(
                                out.ap()[c * P : (c + 1) * P, ns * SL : (ns + 1) * SL],
                                u_sb[:],
                            )

    nc.compile()
    return nc


_NC = None


def kernel(x: np.ndarray, W: np.ndarray, b: np.ndarray) -> np.ndarray:
    global _NC
    if _NC is None:
        _NC = build_nc()

    x = np.ascontiguousarray(x, dtype=np.float32)
    W = np.ascontiguousarray(W, dtype=np.float32)
    b = np.ascontiguousarray(b, dtype=np.float32)

    wT = np.ascontiguousarray(W.T)                      # [768, 2304]
    bcol = np.ascontiguousarray(b[: 2 * H].reshape(2 * ND, P).T)  # [128, 12]
    bvrow = np.ascontiguousarray(b[2 * H :].reshape(1, H))

    in_maps = []
    for i in range(B):
        in_maps.append(
            {
                "xT": np.ascontiguousarray(x[i].T),     # [768, 2048]
                "wT": wT,
                "bcol": bcol,
                "bvrow": bvrow,
            }
        )

    res = run_bass_kernel_spmd(_NC, in_maps, core_ids=list(range(B)))
    return np.stack(
        [np.ascontiguousarray(res.results[i]["out"].T) for i in range(B)], axis=0
    )


# revision 17
# speedup vs baseline: 1.2111x; 1.2111x over previous
"""Single-head attention (no 1/sqrt(d) scaling) for Trainium2, 8 NeuronCores.

Problem: x [8, 2048, 768], W [2304, 768], b [2304]
    qkv = x @ W.T + b ; q,k,v = split(qkv)
    out = softmax(q @ k.T) @ v            -> [8, 2048, 768] fp32

Sharding: data-parallel over batch, one batch element per core. Inputs are
host-transposed (xT [768,2048], wT [768,2304]); the kernel emits out^T
[768, 2048] and the host transposes back during the gather.

Projections and the QK^T logits run in fp32r (full PE rate, ~1.5e-4 rel
rounding — bf16 upstream of the softmax blows up the logit error). The
post-softmax P tiles and v are bf16 (~2e-3 end-to-end rel err, well under
the 2e-2 gate): downstream of exp the quantization no longer amplifies,
and bf16 stationaries get fast weight loads.

Phase A (k/v projection), looped over 512-wide n-slices of x streaming
through double-buffered SBUF slots, W resident:
    kT = (x @ Wk.T + bk).T  transposed layout [h, n]. Resident.
    v  = x @ Wv.T + bv      natural layout, bf16. Resident.
Phase B (attention) per 512-wide n-slice; no max subtraction (|logits| <~60
<< 88 so exp stays in fp32 range; denominators handled unnormalized):
    qT strip = (x @ Wq.T + bq).T  projected on the fly
    S^T[m,n] = k qT       (PSUM, 6 accumulating matmuls per m-chunk)
    P = exp(S^T)          (ACT, rounds to bf16)
    U^T += v_m^T @ P      (6 matmuls, accumulated over 16 m-chunks in 6 banks)
    racc += P             (DVE elementwise accumulate of the 16 P tiles)
    r = ones128 @ racc    (ONE replicated-denominator matmul per slice —
                           replaces the per-m-chunk ones matmul, saving 15
                           512-col PE streams per slice)
    out^T slice = U^T * (1/r)  (approx-reciprocal + DVE scale, DMA to DRAM)
The m-loop is software-pipelined (S/exp for chunk i issued ahead of
racc/U for chunk i-1); S tiles double-buffer through 2 PSUM banks, U 6.

Startup: the PE idles ~10us waiting for the first W/x DMAs while the HAM
clock gate holds it at 1.2 GHz. A dozen dummy matmuls on a memset tile
warm the clock during the DMA window, and the wk/x slice-0 loads are
interleaved so the first real matmul unlocks after ~0.5 MB instead of
~2.7 MB.
"""

import contextlib

import numpy as np

import concourse.bacc as bacc
import concourse.mybir as mybir
import concourse.tile as tile
from concourse.bass_utils import run_bass_kernel_spmd

F32 = mybir.dt.float32
F32R = mybir.dt.float32r
BF16 = mybir.dt.bfloat16
AF = mybir.ActivationFunctionType
ALU = mybir.AluOpType

B, N, H = 8, 2048, 768
H3 = 3 * H
P = 128
ND = H // P      # 6 d-chunks
NM = N // P      # 16 m-chunks
SL = 512         # n-slice width (fp32 moving-operand max / one PSUM bank)
NSL = N // SL    # 4 n-slices


def build_nc(loop_iters=None, split=1, pv_bf16=True, STORE_GP=True, WARM_MMS=16,
             fast_recip=True):
    """Build the attention kernel. loop_iters wraps the whole body in an
    on-device For_i loop (benchmarking only — amortizes dispatch overhead)."""
    nc = bacc.Bacc("TRN2", target_bir_lowering=False, debug=False)

    xT = nc.dram_tensor("xT", [H, N], F32R, kind="ExternalInput")
    wT = nc.dram_tensor("wT", [H, H3], F32R, kind="ExternalInput")
    bcol = nc.dram_tensor("bcol", [P, 2 * ND], F32, kind="ExternalInput")
    bvrow = nc.dram_tensor("bvrow", [1, H], F32, kind="ExternalInput")
    out = nc.dram_tensor("out", [H, N], F32, kind="ExternalOutput")  # transposed; host fixes layout

    def mm_group(psum, lhs_list, rhs_slicer, split=1):
        """Accumulating matmul group into `psum` [P, width]."""
        width = psum.shape[-1]
        hw = width // split
        n = len(lhs_list)
        steps = [(c, h) for c in range(n) for h in range(split)]
        for idx, (c, h) in enumerate(steps):
            lo = h * hw
            nc.tensor.matmul(
                psum[:, lo : lo + hw], lhs_list[c], rhs_slicer(c, lo, hw),
                start=(idx == 0), stop=(idx == len(steps) - 1),
            )

    pdt = BF16 if pv_bf16 else F32R

    with tile.TileContext(nc) as tc:
        with (
            tc.tile_pool(name="dram", bufs=1, space="DRAM") as dram,
            tc.tile_pool(name="const", bufs=1) as const,
            tc.tile_pool(name="keep", bufs=1) as keep,
            tc.For_i(0, loop_iters, 1) if loop_iters else contextlib.nullcontext(),
        ):
            # ---- HAM warmup: dummy matmuls while the first DMAs stream ----
            warm = const.tile([P, SL], F32, name="warm")
            nc.vector.memset(warm[:], 0.0)
            if WARM_MMS:
                warm_ps = tc.alloc_tile_pool(name="warm_ps", bufs=1, space="PSUM")
                wps = warm_ps.tile([P, SL], F32, name="wps")
                for _ in range(WARM_MMS):
                    nc.tensor.matmul(
                        wps[:], warm[:, 0:P].bitcast(F32R), warm[:].bitcast(F32R),
                        start=True, stop=True,
                    )
                warm_ps.release()

            bcol_sb = const.tile([P, 2 * ND], F32)
            ones_f32 = const.tile([P, P], F32, name="ones_f32")
            nc.vector.memset(ones_f32[:], 1.0)
            ones_f32r = ones_f32[:].bitcast(F32R)

            # resident across phases
            ktsb = [keep.tile([P, N], F32R, name=f"kT{c}") for c in range(ND)]
            vsb = [keep.tile([P, H], pdt, name=f"v{ni}") for ni in range(NM)]

            with tc.tile_pool(name="xw_pool", bufs=1) as xw:
                # W resident. q/k sections as [128,128] h-slices so compute
                # unlocks at DMA-stream granularity; v as [128, 768].
                HH = H // 2
                wq = [
                    [xw.tile([P, HH], F32R, name=f"wq{c}_{h}") for h in range(2)]
                    for c in range(ND)
                ]

                def wslice(blks, c, hc):
                    half, col = divmod(hc * P, HH)
                    return blks[c][half][:, col : col + P]
                xwa = tc.alloc_tile_pool(name="xwa_pool", bufs=1)
                wk = [
                    [xwa.tile([P, HH], F32R, name=f"wk{c}_{h}") for h in range(2)]
                    for c in range(ND)
                ]
                wv = [xwa.tile([P, H], F32R, name=f"wv{c}") for c in range(ND)]
                # x is resident for the whole kernel as 2 pairs of
                # [128, 1024] tiles per d-chunk (4KB DMA rows stream the
                # rings at better efficiency than 2KB; phase B reuses the
                # tiles, eliminating the 6MB reload).
                XW = 2 * SL
                xres = [
                    [xw.tile([P, XW], F32R, name=f"x{c}_{pr}") for pr in range(2)]
                    for c in range(ND)
                ]

                def xsl(ns, c, lo, w):
                    return xres[c][ns // 2][:, (ns % 2) * SL + lo : (ns % 2) * SL + lo + w]

                def load_x(pr, lo, w, interleave_with=None):
                    for c in range(ND):
                        if interleave_with is not None:
                            interleave_with(c)
                        nc.sync.dma_start(
                            xres[c][pr][:, lo : lo + w],
                            xT.ap()[
                                c * P : (c + 1) * P,
                                pr * XW + lo : pr * XW + lo + w,
                            ],
                        )

                def load_w_half(blks, lo, h, c_only=None):
                    for c in range(ND) if c_only is None else [c_only]:
                        nc.sync.dma_start(
                            blks[c][h][:],
                            wT.ap()[c * P : (c + 1) * P, lo + h * HH : lo + (h + 1) * HH],
                        )

                bvb = xwa.tile([P, H], F32, name="bvb")

                # DMA order = compute-unlock order (phase A consumes
                # kproj(ns0) -> vproj(ns0) -> kproj(ns1) -> ...):
                #   [wk half0 || x slice0]  -> kproj ns0 hc0-2
                #   wk half1               -> kproj ns0 hc3-5
                #   wv                     -> vproj ns0
                #   x slice1               -> kproj ns1
                #   x pair1                -> slices 2-3
                #   wq                     -> phase B
                load_x(0, 0, SL, interleave_with=lambda c: load_w_half(wk, H, 0, c))
                nc.sync.dma_start(bcol_sb[:], bcol.ap())
                nc.sync.dma_start(bvb[:1, :], bvrow.ap())
                nc.gpsimd.partition_broadcast(bvb[:], bvb[:1, :])
                load_w_half(wk, H, 1)
                for c in range(ND):
                    nc.sync.dma_start(
                        wv[c][:], wT.ap()[c * P : (c + 1) * P, 2 * H : 3 * H]
                    )
                load_x(0, SL, SL)
                load_x(1, 0, XW)
                load_w_half(wq, 0, 0)
                load_w_half(wq, 0, 1)

                with (
                    tc.tile_pool(name="qkps", bufs=3, space="PSUM") as qkps,
                    tc.tile_pool(name="vps", bufs=2, space="PSUM") as vps,
                ):
                    for ns in range(NSL):
                        ssl = slice(ns * SL, (ns + 1) * SL)

                        # --- k projection for this slice (resident) ---
                        for hc in range(ND):
                            ps = qkps.tile([P, SL], F32, name="qkpsum", tag="qk")
                            mm_group(
                                ps, [wslice(wk, c, hc) for c in range(ND)],
                                lambda c, lo, w, _ns=ns: xsl(_ns, c, lo, w),
                                split=split,
                            )
                            nc.scalar.activation(
                                ktsb[hc][:, ssl], ps[:], AF.Identity,
                                bias=bcol_sb[:, ND + hc : ND + hc + 1],
                            )

                        # --- v projection for the 4 n-chunks of this slice ---
                        for ni in range(4 * ns, 4 * ns + 4):
                            lof = (ni % NSL) * P
                            pa = vps.tile([P, SL], F32, name="pa", tag="pa")
                            pb = vps.tile([P, H - SL], F32, name="pb", tag="pb")
                            mm_group(
                                pa, [xsl(ns, c, lof, P) for c in range(ND)],
                                lambda c, lo, w: wv[c][:, lo : lo + w],
                                split=split,
                            )
                            mm_group(
                                pb, [xsl(ns, c, lof, P) for c in range(ND)],
                                lambda c, lo, w: wv[c][:, SL + lo : SL + lo + w],
                            )
                            nc.vector.tensor_tensor(
                                vsb[ni][:, 0:SL], pa[:], bvb[:, 0:SL], op=ALU.add
                            )
                            nc.vector.tensor_tensor(
                                vsb[ni][:, SL:H], pb[:], bvb[:, SL:H], op=ALU.add
                            )

                xwa.release()

                # ---- Phase B: attention (software-pipelined m-loop) ----
                with (
                    tc.tile_pool(name="qsb_pool", bufs=2) as qsb_pool,
                    tc.tile_pool(name="p_pool", bufs=4) as p_pool,
                    tc.tile_pool(name="u_ps", bufs=1, space="PSUM") as u_ps,
                    tc.tile_pool(name="sps", bufs=2, space="PSUM") as sps,
                    tc.tile_pool(name="usb_pool", bufs=1) as usb_pool,
                    tc.tile_pool(name="misc", bufs=1) as misc,
                ):
                    for ns in range(NSL):
                        # project this slice's q strip (transposed layout)
                        qsbuf = []
                        for hc in range(ND):
                            ps = sps.tile([P, SL], F32, name="s_ps", tag="s")
                            mm_group(
                                ps, [wslice(wq, c, hc) for c in range(ND)],
                                lambda c, lo, w, _ns=ns: xsl(_ns, c, lo, w),
                                split=split,
                            )
                            qc = qsb_pool.tile([P, SL], F32R, name=f"qsb{hc}", tag=f"qsb{hc}")
                            nc.scalar.activation(
                                qc[:], ps[:], AF.Identity, bias=bcol_sb[:, hc : hc + 1]
                            )
                            qsbuf.append(qc)
                        us = [
                            u_ps.tile([P, SL], F32, name=f"u{c}", tag=f"u{c}")
                            for c in range(ND)
                        ]
                        racc = misc.tile([P, SL], F32R, name="racc", tag="racc")

                        p_sbs = [None] * NM
                        for mi in range(NM + 1):
                            if mi < NM:
                                msl = slice(mi * P, (mi + 1) * P)
                                s_ps = sps.tile([P, SL], F32, name="s_ps", tag="s")
                                mm_group(
                                    s_ps, [ktsb[c][:, msl] for c in range(ND)],
                                    lambda c, lo, w: qsbuf[c][:, lo : lo + w],
                                    split=split,
                                )
                                p_sb = p_pool.tile([P, SL], pdt, name="p_sb", tag="p")
                                nc.scalar.activation(p_sb[:], s_ps[:], AF.Exp)
                                p_sbs[mi] = p_sb
                            if mi >= 1:
                                j = mi - 1
                                pj = p_sbs[j]
                                if j == 0:
                                    nc.vector.tensor_copy(racc[:], pj[:])
                                else:
                                    nc.vector.tensor_tensor(
                                        racc[:], pj[:], racc[:], op=ALU.add
                                    )
                                for c in range(ND):
                                    hw2 = SL // split
                                    for h in range(split):
                                        lo = h * hw2
                                        nc.tensor.matmul(
                                            us[c][:, lo : lo + hw2],
                                            vsb[j][:, c * P : (c + 1) * P],
                                            pj[:, lo : lo + hw2],
                                            start=(j == 0 and h == 0),
                                            stop=(j == NM - 1 and h == split - 1),
                                        )
                                p_sbs[j] = None

                        # one replicated-denominator matmul for the whole slice
                        r_ps = sps.tile([P, SL], F32, name="r_ps", tag="s")
                        nc.tensor.matmul(
                            r_ps[:], ones_f32r, racc[:], start=True, stop=True
                        )
                        rinv = misc.tile([P, SL], F32, name="rinv", tag="rinv")
                        if fast_recip:
                            nc.vector.reciprocal_approx_fast(rinv[:], r_ps[:])
                        else:
                            nc.vector.reciprocal(rinv[:], r_ps[:])

                        for c in range(ND):
                            u_sb = usb_pool.tile([P, SL], F32, name=f"usb{c}", tag=f"usb{c}")
                            nc.vector.tensor_tensor(u_sb[:], us[c][:], rinv[:], op=ALU.mult)
                            store_eng = nc.gpsimd if STORE_GP else nc.sync
                            store_eng.dma_start(
                                out.ap()[c * P : (c + 1) * P, ns * SL : (ns + 1) * SL],
                                u_sb[:],
                            )

    nc.compile()
    return nc


_NC = None


def kernel(x: np.ndarray, W: np.ndarray, b: np.ndarray) -> np.ndarray:
    global _NC
    if _NC is None:
        _NC = build_nc()

    x = np.ascontiguousarray(x, dtype=np.float32)
    W = np.ascontiguousarray(W, dtype=np.float32)
    b = np.ascontiguousarray(b, dtype=np.float32)

    wT = np.ascontiguousarray(W.T)                      # [768, 2304]
    bcol = np.ascontiguousarray(b[: 2 * H].reshape(2 * ND, P).T)  # [128, 12]
    bvrow = np.ascontiguousarray(b[2 * H :].reshape(1, H))

    in_maps = []
    for i in range(B):
        in_maps.append(
            {
                "xT": np.ascontiguousarray(x[i].T),     # [768, 2048]
                "wT": wT,
                "bcol": bcol,
                "bvrow": bvrow,
            }
        )

    res = run_bass_kernel_spmd(_NC, in_maps, core_ids=list(range(B)))
    return np.stack(
        [np.ascontiguousarray(res.results[i]["out"].T) for i in range(B)], axis=0
    )
